# revision 1
# baseline (speedup 1.0000x reference)
"""AttnBlock2D (B=4, C=512, H=W=64) on 8 Trainium2 NeuronCores.

Strategy: data-parallel over batch x sequence-parallel over output tokens.
Core c handles image b = c//2 and output-token half h = c%2 (2048 of 4096
tokens).  Attention runs in the "scores-transposed" formulation (softmax
axis j on SBUF partitions, zero on-chip transposes) with the score bilinear
form factored on the host:

    scores[i,j] = (Wk x_i + bk).(Wq x_j + bq)
                = x_i^T (Wk^T Wq) x_j + bk.(Wq x_j) + [i-only terms]

The i-only terms cancel in softmax over j, so the kernel computes
    h           = (Wk^T Wq) x      (one GEMM; the lhsT input carries Wq^T Wk)
    t[j]        = SCALE * (Wq^T bk) . x_j   (N=2 matmuls sharing the vT
                                             GEMM's stationary x chunks)
    e^T[j, i]   = exp(SCALE * h_j . x_i + t[j])   (ScalarE, per-partition
                                                   bias, no max-subtract)
    acc[p, i]   = sum_jc e^T[jc*128+p, i]         (DVE partial-sum)
    s[i]        = 1^T @ acc                       (one matmul per i-block)
    u[c, i]     = sum_j vT[j, c] e^T[j, i]        (lhsT = vT, rhs = e^T)
    y[co, i]    = (Wo @ u)[co, i] / s[i] + bo'[co]

so k is NEVER materialised (the scores matmul reads raw x), q's bias
cancels entirely, v's bias folds into bo' = Wo @ bv + bo on the host, and
normalisation is deferred through the projection.  All matmuls use
float32r (FP22 multiply, fp32 accumulate), full PE rate at free dim >= 256.
"""

import numpy as np

import concourse.bass as bass
import concourse.tile as tile
import concourse.mybir as mybir
from concourse import bacc
from concourse.bass_utils import run_bass_kernel_spmd

B = 4
C = 512            # C_IN == C_HID
HW = 64 * 64       # tokens per image
NCORES = 8
I = HW * B // NCORES   # 2048 output tokens per core

CK = 128           # partition chunk
NB = 512           # free-dim block
NCH = C // CK      # 4
NJB = HW // CK     # 32
NIB = I // NB      # 4
NNB = HW // NB     # 8

F32 = mybir.dt.float32
F32R = mybir.dt.float32r
AF = mybir.ActivationFunctionType
SCALE = 1.0 / float(np.sqrt(float(C)))


def build_bass(reps=1):
    nc = bacc.Bacc(
        "TRN2", target_bir_lowering=False, debug=False, enable_asserts=False
    )

    x = nc.dram_tensor("x", [C, HW], F32R, kind="ExternalInput").ap()
    xi = nc.dram_tensor("xi", [C, I], F32R, kind="ExternalInput").ap()
    wqT = nc.dram_tensor("wqT", [C, C], F32R, kind="ExternalInput").ap()
    wkT = nc.dram_tensor("wkT", [C, C], F32R, kind="ExternalInput").ap()
    wvT = nc.dram_tensor("wvT", [C, C], F32R, kind="ExternalInput").ap()
    bqp = nc.dram_tensor("bqp", [CK, NCH], F32, kind="ExternalInput").ap()
    bkp = nc.dram_tensor("bkp", [CK, NCH, 2], F32R,
                         kind="ExternalInput").ap()
    bop = nc.dram_tensor("bop", [CK, NCH], F32, kind="ExternalInput").ap()
    onesd = nc.dram_tensor("onesd", [CK, 1], F32R, kind="ExternalInput").ap()
    out = nc.dram_tensor("out", [C, I], F32R, kind="ExternalOutput").ap()

    # DRAM views with the channel dim split for 128-partition DMA
    x3 = x.rearrange("(a p) n -> p a n", p=CK)      # [128, 4, 4096]
    xi3 = xi.rearrange("(a p) n -> p a n", p=CK)    # [128, 4, 2048]
    wq3d = wqT.rearrange("(a p) n -> p a n", p=CK)  # [128, 4, 512]
    wk3d = wkT.rearrange("(a p) n -> p a n", p=CK)
    wv3d = wvT.rearrange("(a p) n -> p a n", p=CK)
    out3 = out.rearrange("(a p) n -> p a n", p=CK)  # [128, 4, 2048]

    with tile.TileContext(nc) as tc:
        with tc.tile_pool(name="persist", bufs=1) as persist, \
             tc.tile_pool(name="xp", bufs=3) as xp, \
             tc.tile_pool(name="ep", bufs=3) as ep, \
             tc.tile_pool(name="ftp", bufs=4) as ftp, \
             tc.tile_pool(name="accp", bufs=1) as accp, \
             tc.tile_pool(name="rp", bufs=1) as rp, \
             tc.tile_pool(name="psA", bufs=4, space="PSUM") as psA, \
             tc.tile_pool(name="psO", bufs=1, space="PSUM") as psO, \
             tc.tile_pool(name="wqv", bufs=1) as wqv:

            # ---- persistent SBUF state ----
            q = persist.tile([CK, NCH, HW], F32R, name="q")    # q[ch, n]
            vT = persist.tile([CK, NJB, C], F32R, name="vT")   # vT[j, c]
            up_t = persist.tile([CK, NCH, 2], F32R, name="up_t")
            tt = persist.tile([CK, NJB], F32, name="tt")
            biases = persist.tile([CK, 3 * NCH], F32, name="biases")
            ones128 = persist.tile([CK, 1], F32R, name="ones128")
            wq3 = wqv.tile([CK, NCH, C], F32R, name="wq3")
            wv3 = wqv.tile([CK, NCH, C], F32R, name="wv3")

            # weights for the first matmuls go first on the SP DMA queue;
            # everything else rides the gpsimd queue so it doesn't delay them
            for ci in range(NCH):
                nc.scalar.dma_start(out=wq3[:, ci, :], in_=wq3d[:, ci, :])
            nc.gpsimd.dma_start(out=biases[:, 0:NCH], in_=bqp)
            nc.gpsimd.dma_start(out=up_t, in_=bkp)
            nc.gpsimd.dma_start(out=biases[:, 2 * NCH:3 * NCH], in_=bop)
            nc.gpsimd.dma_start(out=ones128, in_=onesd)
            nc.gpsimd.dma_start(out=wv3, in_=wv3d)

            bq_col = lambda cc: biases[:, cc:cc + 1]
            bo_col = lambda cc: biases[:, 2 * NCH + cc:2 * NCH + cc + 1]

            # ---- phase B: q (all tokens) and vT (all tokens) ----
            rep_ctx = tc.For_i(0, reps, 1) if reps > 1 else None
            if rep_ctx is not None:
                rep_ctx.__enter__()
            for nb in range(NNB):
                xt = xp.tile([CK, NCH, NB], F32R, name="xt", tag="xt")
                xt_ci = lambda ci: xt[:, ci, :]
                if nb == 0:
                    for ci in range(NCH):
                        nc.sync.dma_start(
                            out=xt_ci(ci),
                            in_=x3[:, ci, nb * NB:(nb + 1) * NB],
                        )
                else:
                    nc.sync.dma_start(
                        out=xt, in_=x3[:, :, nb * NB:(nb + 1) * NB]
                    )
                # q[ch, nb-block] += wqT[cin, ch].T @ x[cin, nb-block]
                for cc in range(NCH):
                    pq = psA.tile([CK, NB], F32, name="pq", tag="psA",
                                  space="PSUM")
                    for ci in range(NCH):
                        nc.tensor.matmul(
                            pq,
                            lhsT=(wq3[:, ci, cc * CK:(cc + 1) * CK]),
                            rhs=(xt_ci(ci)),
                            start=(ci == 0), stop=(ci == NCH - 1),
                        )
                    nc.vector.tensor_scalar_add(
                        q[:, cc, nb * NB:(nb + 1) * NB], pq, bq_col(cc)
                    )
                # vT[nb-block, ch] += x[cin, j-chunk].T @ wvT[cin, ch] (+bv)
                for nn in range(NB // CK):
                    jc = nb * (NB // CK) + nn
                    pv = psA.tile([CK, C], F32, name="pv", tag="psA",
                                  space="PSUM")
                    for ci in range(NCH):
                        nc.tensor.matmul(
                            pv,
                            lhsT=(xt_ci(ci)[:, nn * CK:(nn + 1) * CK]),
                            rhs=(wv3[:, ci, :]),
                            start=(ci == 0), stop=(ci == NCH - 1),
                        )
                    nc.vector.tensor_copy(vT[:, jc, :], pv)
                    # t[j] = SCALE * (Wq^T bk) . x_j  (per-j softmax bias)
                    pt = psA.tile([CK, 2], F32, name="pt", tag="psA",
                                  space="PSUM")
                    for ci in range(NCH):
                        nc.tensor.matmul(
                            pt,
                            lhsT=(xt_ci(ci)[:, nn * CK:(nn + 1) * CK]),
                            rhs=(up_t[:, ci, :]),
                            start=(ci == 0), stop=(ci == NCH - 1),
                        )
                    nc.vector.tensor_copy(tt[:, jc:jc + 1], pt[:, 0:1])

            # ---- phase C: k, attention, projection per 512-token i-block ----
            def k_block(ib):
                # xi tile for i-block ib (raw x feeds the scores matmul now)
                xt2 = xp.tile([CK, NCH, NB], F32R, name="xt2", tag="xt")
                nc.sync.dma_start(
                    out=xt2, in_=xi3[:, :, ib * NB:(ib + 1) * NB]
                )
                return xt2

            kt = k_block(0)
            for ib in range(NIB):
                po = [
                    psO.tile([CK, NB], F32, name=f"po{cc}", tag=f"po{cc}",
                             space="PSUM")
                    for cc in range(NCH)
                ]
                psum = psA.tile([1, NB], F32, name="psum", tag="psA",
                                space="PSUM")
                acc = accp.tile([CK, NB], F32R, name="acc", tag="acc")
                for jc in range(NJB):
                    ps_ = psA.tile([CK, NB], F32, name="ps", tag="psA",
                                   space="PSUM")
                    for cc in range(NCH):
                        nc.tensor.matmul(
                            ps_,
                            lhsT=(q[:, cc, jc * CK:(jc + 1) * CK]),
                            rhs=(kt[:, cc, :]),  # kt = raw xi tile
                            start=(cc == 0), stop=(cc == NCH - 1),
                        )
                    et = ep.tile([CK, NB], F32R, name="et", tag="et")
                    nc.scalar.activation(et, ps_, AF.Exp, scale=SCALE,
                                         bias=tt[:, jc:jc + 1])
                    # partial softmax denominators accumulate on DVE
                    if jc == 0:
                        nc.vector.tensor_copy(acc, et)
                    else:
                        nc.vector.tensor_add(acc, acc, et)
                    # apply: u[c, i] += vT[j, c].T @ e^T[j, i]
                    for cc in range(NCH):
                        nc.tensor.matmul(
                            po[cc],
                            lhsT=(vT[:, jc, cc * CK:(cc + 1) * CK]),
                            rhs=(et),
                            start=(jc == 0), stop=(jc == NJB - 1),
                        )
                if ib + 1 < NIB:
                    next_kt = k_block(ib + 1)
                # single cross-partition reduce: s[i] = 1^T @ acc
                nc.tensor.matmul(psum, lhsT=(ones128), rhs=(acc),
                                 start=True, stop=True)
                # normalisation factors r[i] = 1 / s[i], broadcast
                r1 = rp.tile([1, NB], F32, name="r1", tag="r1")
                nc.vector.reciprocal(r1, psum)
                rb = rp.tile([CK, NB], F32, name="rb", tag="rb")
                nc.gpsimd.partition_broadcast(rb, r1)
                # apply psums already hold Wo @ u (weights carry
                # (Wo Wv)^T): normalise, add bias, store
                last = ib == NIB - 1
                for cc in range(NCH):
                    splits = ((0, NB // 2), (NB // 2, NB)) if last and \
                        cc == NCH - 1 else ((0, NB),)
                    for lo, hi in splits:
                        if cc == 0 and lo == 0:
                            ft = acc[:, 0:NB]
                        else:
                            ft = ftp.tile([CK, hi - lo], F32R, name="ft",
                                          tag="ft")
                        nc.vector.tensor_mul(ft, po[cc][:, lo:hi],
                                             rb[:, lo:hi])
                        nc.vector.tensor_scalar_add(ft, ft, bo_col(cc))
                        nc.sync.dma_start(
                            out=out3[:, cc, ib * NB + lo:ib * NB + hi],
                            in_=ft,
                        )
                if ib + 1 < NIB:
                    kt = next_kt
            if rep_ctx is not None:
                rep_ctx.__exit__(None, None, None)

    nc.compile()
    return nc


_NC = None


def _get_nc():
    global _NC
    if _NC is None:
        _NC = build_bass()
    return _NC


def _make_in_maps(inp, Wk, bk, Wq, bq, Wv, bv, Wo, bo):
    x_all = np.ascontiguousarray(
        np.asarray(inp, dtype=np.float32).reshape(B, C, HW)
    )
    wqT = np.ascontiguousarray(
        (np.asarray(Wq, np.float64).T @ np.asarray(Wk, np.float64))
        .astype(np.float32))
    wkT = np.ascontiguousarray(np.asarray(Wk, np.float32).T)
    wvT = np.ascontiguousarray(
        (np.asarray(Wo, np.float64) @ np.asarray(Wv, np.float64))
        .T.astype(np.float32))
    # biases packed [128, 4] so column cc is the per-partition bias of channel
    # chunk cc
    # scores bilinear form precomputed on host: wqT slot carries
    # (Wq^T Wk) so the "q" GEMM produces h = (Wk^T Wq) x; softmax cancels
    # all i-only bias terms, and the j-dependent term bk.(Wq x_j) rides the
    # exp bias via u' = SCALE * Wq^T bk (in the bkp slot). bq cancels fully.
    bqp = np.zeros((CK, NCH), np.float32)
    u_eff = SCALE * (np.asarray(Wq, np.float64).T @ np.asarray(bk, np.float64))
    up2 = np.zeros((CK, NCH, 2), np.float32)
    up2[:, :, 0] = u_eff.astype(np.float32).reshape(NCH, CK).T
    bkp = np.ascontiguousarray(up2)
    bo_eff = (np.asarray(Wo, np.float32) @ np.asarray(bv, np.float32)
              + np.asarray(bo, np.float32))
    bop = np.ascontiguousarray(bo_eff.reshape(NCH, CK).T)
    onesd = np.ones((CK, 1), np.float32)

    in_maps = []
    for c in range(NCORES):
        b, h = divmod(c, NCORES // B)
        in_maps.append({
            "x": x_all[b],
            "xi": np.ascontiguousarray(x_all[b][:, h * I:(h + 1) * I]),
            "wqT": wqT, "wkT": wkT, "wvT": wvT,
            "bqp": bqp, "bkp": bkp, "bop": bop,
            "onesd": onesd,
        })
    return in_maps


def run(trace=False, tmpdir=None, **inputs):
    nc = _get_nc()
    in_maps = _make_in_maps(**inputs)
    res = run_bass_kernel_spmd(
        nc, in_maps, core_ids=list(range(NCORES)), trace=trace, tmpdir=tmpdir
    )
    full = np.empty((B, C, HW), dtype=np.float32)
    for c in range(NCORES):
        b, h = divmod(c, NCORES // B)
        full[b][:, h * I:(h + 1) * I] = res.results[c]["out"]
    return full.reshape(B, C, 64, 64), res


def kernel(**inputs):
    out, _ = run(trace=False, **inputs)
    return out



# revision 40
# speedup vs baseline: 1.0746x; 1.0746x over previous
"""AttnBlock2D (B=4, C=512, H=W=64) on 8 Trainium2 NeuronCores.

Strategy: data-parallel over batch x sequence-parallel over output tokens.
Core c handles image b = c//2 and output-token half h = c%2.  The host
permutes the token axis per core (own half first), so the SPMD program is
identical on every core.

Math (scores-transposed formulation, softmax axis j on partitions).  With
M = Wk^T Wq and u = Wq^T bk, the softmax-relevant part of the score is

    score[i,j] = x_i^T M x_j + u . x_j  = g_i . x_j,   g_i = M^T x_i + u

(bq and all i-only terms cancel in softmax).  So the kernel computes

    g           = M^T x + u            (GEMM over the core's i-half ONLY;
                                        the j-side operand is raw x)
    e^T[j, i]   = exp(SCALE * g_i . x_j)     (ScalarE, bf16 out)
    acc[p, i]   = sum_jc e^T[jc*128+p, i]    (DVE partial-sum, f32)
    s[i]        = 1^T @ acc                  (one matmul per i-block)
    u[c, i]     = sum_j vT'[j, c] e^T[j, i]  (lhsT = vT', rhs = e^T)
    y[c, i]     = u[c, i] / s[i]

with vT'[j, c] = ((Wo Wv) x_j)_c + (Wo bv + bo)_c: softmax weights sum to
one, so the whole output bias (and Wo itself) ride inside vT and the
epilogue is a single normalising multiply.  All matmul operands are
bfloat16 (full PE rate at any free dim, half the DMA/SBUF of f32);
accumulation stays f32.

Schedule: warm-up matmuls on a memset scratch absorb the PE p-state ramp
during the initial weight DMA; apply matmuls trail the scores by a few
j-chunks so each i-block's epilogue (sum/recip/broadcast/mul/store) hides
under the next block's score matmuls; final i-blocks narrow down
(512,512,512,256,128,128) so the last epilogue tail is short.
"""

import numpy as np

import concourse.bass as bass  # noqa: F401  (engine types via nc.*)
import concourse.tile as tile
import concourse.mybir as mybir
from concourse import bacc
from concourse.alu_op_type import AluOpType
from concourse.bass_utils import run_bass_kernel_spmd

B = 4
C = 512            # C_IN == C_HID
HW = 64 * 64       # tokens per image
NCORES = 8
I = HW * B // NCORES   # 2048 output tokens per core

CK = 128           # partition chunk
NB = 512           # x-tile token width
NCH = C // CK      # 4 channel chunks
NJB = HW // CK     # 32 j-chunks
NNB = HW // NB     # 8 x tiles
NIB_MINE = I // NB  # 4 x tiles in my half

BLOCKS = [512, 512, 512, 256, 128, 128]   # i-block widths (sum = I)
assert sum(BLOCKS) == I

F32 = mybir.dt.float32
F32R = mybir.dt.float32r
BF16 = mybir.dt.bfloat16
AF = mybir.ActivationFunctionType
SCALE = 1.0 / float(np.sqrt(float(C)))

N_WARMUP = 12      # dummy matmuls riding out the first weight DMA


def build_bass():
    nc = bacc.Bacc(
        "TRN2", target_bir_lowering=False, debug=False, enable_asserts=False
    )

    xb = nc.dram_tensor("xb", [C, HW], BF16, kind="ExternalInput").ap()
    wqT = nc.dram_tensor("wqT", [C, C], BF16, kind="ExternalInput").ap()
    wvT = nc.dram_tensor("wvT", [C, C], BF16, kind="ExternalInput").ap()
    ub = nc.dram_tensor("ub", [CK, NCH], F32, kind="ExternalInput").ap()
    borow = nc.dram_tensor("borow", [1, C], F32, kind="ExternalInput").ap()
    out = nc.dram_tensor("out", [C, I], BF16, kind="ExternalOutput").ap()

    x3 = xb.rearrange("(a p) n -> p a n", p=CK)     # [128, 4, 4096]
    wq3d = wqT.rearrange("(a p) n -> p a n", p=CK)  # [128, 4, 512]
    wv3d = wvT.rearrange("(a p) n -> p a n", p=CK)
    out3 = out.rearrange("(a p) n -> p a n", p=CK)  # [128, 4, 2048]

    with tile.TileContext(nc) as tc:
        with tc.tile_pool(name="persist", bufs=1) as persist, \
             tc.tile_pool(name="ep", bufs=16) as ep, \
             tc.tile_pool(name="accp", bufs=2) as accp, \
             tc.tile_pool(name="rp", bufs=2) as rp, \
             tc.tile_pool(name="ftp", bufs=2) as ftp, \
             tc.tile_pool(name="psA", bufs=3, space="PSUM") as psA, \
             tc.tile_pool(name="psS", bufs=1, space="PSUM") as psS, \
             tc.tile_pool(name="psO", bufs=1, space="PSUM") as psO:

            # ---- persistent SBUF state ----
            x = persist.tile([CK, NCH, HW], BF16, name="x")   # all tokens
            g = persist.tile([CK, NCH, I], BF16, name="g")    # my i-half
            vT = persist.tile([CK, NJB, C], BF16, name="vT")
            ubias = persist.tile([CK, NCH], F32, name="ubias")
            bo_b = persist.tile([CK, C], F32, name="bo_b")
            bo_r = persist.tile([1, C], F32, name="bo_r")
            ones128 = persist.tile([CK, 1], F32, name="ones128")
            wq3 = persist.tile([CK, NCH, C], BF16, name="wq3")
            wv3 = persist.tile([CK, NCH, C], BF16, name="wv3")
            scratch = persist.tile([CK, NB], F32, name="scratch")

            # ---- input DMAs, first-needed first per queue ----
            # scalar queue: wq (first g matmul), then odd x tiles
            nc.scalar.dma_start(out=wq3, in_=wq3d)
            # sync queue: x tile nb=0, then even tiles
            nc.sync.dma_start(out=x[:, :, 0:NB], in_=x3[:, :, 0:NB])
            # gpsimd queue: small tensors, then wv (phase B's v GEMMs)
            nc.gpsimd.dma_start(out=ubias, in_=ub)
            nc.gpsimd.dma_start(out=bo_r, in_=borow)
            for ci in range(NCH):
                nc.gpsimd.dma_start(out=wv3[:, ci, :], in_=wv3d[:, ci, :])
            for nb in range(1, NNB):
                eng = nc.sync if nb % 2 == 0 else nc.scalar
                eng.dma_start(out=x[:, :, nb * NB:(nb + 1) * NB],
                              in_=x3[:, :, nb * NB:(nb + 1) * NB])

            # ---- warm-up: keep PE busy (and the p-state ramp burning)
            # while the first weight/x DMAs land; results land in a PSUM
            # bank that the first real accumulation group overwrites ----
            nc.gpsimd.memset(scratch, 0.0)
            nc.gpsimd.memset(ones128, 1.0)
            nc.gpsimd.partition_broadcast(bo_b, bo_r)
            psD = psS.tile([CK, NB], F32, name="psD", tag="sum",
                           space="PSUM")
            scr_r = scratch.bitcast(F32R)
            for i in range(N_WARMUP):
                # one accumulation group: no inter-matmul semaphores
                nc.tensor.matmul(psD, lhsT=scr_r[:, 0:CK], rhs=scr_r,
                                 start=(i == 0), stop=(i == N_WARMUP - 1))

            # ---- phase B: g over my half first (needs only wq + x0..3,
            # matching DMA arrival order), then vT' for all tokens ----
            for nb in range(NIB_MINE):
                for cc in range(NCH):
                    pg = psA.tile([CK, NB], F32, name="pg", tag="psA",
                                  space="PSUM")
                    for ci in range(NCH):
                        nc.tensor.matmul(
                            pg,
                            lhsT=wq3[:, ci, cc * CK:(cc + 1) * CK],
                            rhs=x[:, ci, nb * NB:(nb + 1) * NB],
                            start=(ci == 0), stop=(ci == NCH - 1),
                        )
                    # g = M^T x + u; bq cancels in softmax
                    nc.scalar.activation(
                        g[:, cc, nb * NB:(nb + 1) * NB], pg, AF.Identity,
                        bias=ubias[:, cc:cc + 1])
            for jc in range(NJB):
                pv = psA.tile([CK, C], F32, name="pv", tag="psA",
                              space="PSUM")
                for ci in range(NCH):
                    nc.tensor.matmul(
                        pv,
                        lhsT=x[:, ci, jc * CK:(jc + 1) * CK],
                        rhs=wv3[:, ci, :],
                        start=(ci == 0), stop=(ci == NCH - 1),
                    )
                # vT' = (WoWv)x + (Wo bv + bo): softmax weights sum to
                # 1, so the output bias rides inside vT
                nc.vector.tensor_tensor(out=vT[:, jc, :], in0=pv,
                                        in1=bo_b, op=AluOpType.add)

            # ---- phase C: attention per i-block, software-pipelined ----
            offs = np.cumsum([0] + BLOCKS).tolist()

            state: dict = {}
            prev: dict = {}
            pending: list = []    # deferred closures from the previous block

            def scores_chunk(jc):
                w, off = state["w"], state["off"]
                ps_ = psA.tile([CK, w], F32, name="ps", tag="psA",
                               space="PSUM")
                for ci in range(NCH):
                    nc.tensor.matmul(
                        ps_,
                        lhsT=x[:, ci, jc * CK:(jc + 1) * CK],
                        rhs=g[:, ci, off:off + w],
                        start=(ci == 0), stop=(ci == NCH - 1),
                    )
                et = ep.tile([CK, w], BF16, name="et", tag="et")
                nc.scalar.activation(et, ps_, AF.Exp, scale=SCALE)
                acc = state["acc"][:, 0:w]
                if jc == 0:
                    nc.vector.tensor_copy(acc, et)
                else:
                    # alternate engines: DVE also runs the epilogue muls
                    # (gpsimd cannot touch PSUM), so share the adds
                    eng = nc.vector if jc % 2 == 0 else nc.gpsimd
                    eng.tensor_add(acc, acc, et)
                state["ets"][jc] = et

            def make_apply(st, jc):
                def go():
                    et = st["ets"][jc]
                    for cc in range(NCH):
                        nc.tensor.matmul(
                            st["po"][cc],
                            lhsT=vT[:, jc, cc * CK:(cc + 1) * CK],
                            rhs=et,
                            start=(jc == 0), stop=(jc == NJB - 1),
                        )
                return go

            def make_sum_chain(st):
                def go():
                    w = st["w"]
                    psum = psS.tile([1, w], F32, name="psum", tag="sum",
                                    space="PSUM")
                    nc.tensor.matmul(psum, lhsT=ones128.bitcast(F32R),
                                     rhs=st["acc"][:, 0:w],
                                     start=True, stop=True)
                    r1 = rp.tile([1, NB], F32, name="r1", tag="r1")
                    nc.vector.reciprocal(r1[:, 0:w], psum)
                    rb = rp.tile([CK, NB], F32, name="rb", tag="rb")
                    nc.gpsimd.partition_broadcast(rb[:, 0:w], r1[:, 0:w])
                    st["rb"] = rb
                return go

            def make_epilogue(st):
                def go():
                    w, off, rb = st["w"], st["off"], st["rb"]
                    ft = ftp.tile([CK, NCH, NB], BF16, name="ft", tag="ft")
                    for cc in range(NCH):
                        # must be DVE: gpsimd cannot read PSUM
                        nc.vector.tensor_tensor(out=ft[:, cc, 0:w],
                                                in0=st["po"][cc],
                                                in1=rb[:, 0:w],
                                                op=AluOpType.mult)
                    # one fused store: a single HWDGE generation on the
                    # shared DGE device instead of four
                    nc.sync.dma_start(out=out3[:, :, off:off + w],
                                      in_=ft[:, :, 0:w])
                return go

            # per-width pipeline knobs: narrow blocks have shorter score
            # chunks, so the fixed-latency epilogue/exp chains need more
            # slots of cover before the applies start
            KNOBS = {512: (5, 3), 256: (8, 5), 128: (13, 9), 64: (22, 16)}
            for k, w in enumerate(BLOCKS):
                start_slot, lag = KNOBS[w]
                state = {
                    "w": w,
                    "off": offs[k],
                    "acc": accp.tile([CK, NB], F32R, name="acc", tag="acc"),
                    "po": [psO.tile([CK, w], F32, name=f"po{cc}",
                                    tag=f"po{cc}", space="PSUM")
                           for cc in range(NCH)],
                    "ets": [None] * NJB,
                }
                nxt = 0          # next own apply chunk to emit
                for jc in range(NJB):
                    scores_chunk(jc)
                    if pending:
                        pending.pop(0)()
                        if not pending and prev:
                            # prev block's po complete: sum/recip/bcast were
                            # emitted at slot 1; normalising muls go now so
                            # they run under the scores-only slots below
                            make_epilogue(prev)()
                    elif jc >= start_slot:
                        # catch up two per slot, then settle at lag
                        for _ in range(2):
                            if nxt <= jc - lag:
                                make_apply(state, nxt)()
                                nxt += 1
                    if jc == 1 and prev:
                        make_sum_chain(prev)()
                # defer the last `lag` apply chunks into the next block
                pending = [make_apply(state, t) for t in range(nxt, NJB)]
                prev = state

            # tail: drain the last block
            for fn in pending:
                fn()
            make_sum_chain(prev)()
            make_epilogue(prev)()

    nc.compile()
    return nc


_NC = None


def _get_nc():
    global _NC
    if _NC is None:
        _NC = build_bass()
    return _NC


def _make_in_maps(inp, Wk, bk, Wq, bq, Wv, bv, Wo, bo):
    import ml_dtypes

    bf = ml_dtypes.bfloat16
    x_all = np.asarray(inp, dtype=np.float32).reshape(B, C, HW)
    # M = Wk^T Wq: lhsT of the g GEMM (g = M^T x + u)
    wqT = np.ascontiguousarray(
        (np.asarray(Wk, np.float64).T @ np.asarray(Wq, np.float64))
    ).astype(bf)
    wvT = np.ascontiguousarray(
        (np.asarray(Wo, np.float64) @ np.asarray(Wv, np.float64)).T
    ).astype(bf)
    u = (np.asarray(Wq, np.float64).T @ np.asarray(bk, np.float64))
    ub = np.ascontiguousarray(
        u.astype(np.float32).reshape(NCH, CK).T)
    bo_eff = (np.asarray(Wo, np.float32) @ np.asarray(bv, np.float32)
              + np.asarray(bo, np.float32))
    borow = np.ascontiguousarray(bo_eff.reshape(1, C).astype(np.float32))

    in_maps = []
    for c in range(NCORES):
        b, h = divmod(c, NCORES // B)
        xb = x_all[b]
        if h == 1:      # own half first; attention is j-permutation-invariant
            xb = np.concatenate([xb[:, I:], xb[:, :I]], axis=1)
        in_maps.append({
            "xb": np.ascontiguousarray(xb).astype(bf),
            "wqT": wqT, "wvT": wvT, "ub": ub, "borow": borow,
        })
    return in_maps


def run(trace=False, tmpdir=None, **inputs):
    nc = _get_nc()
    in_maps = _make_in_maps(**inputs)
    res = run_bass_kernel_spmd(
        nc, in_maps, core_ids=list(range(NCORES)), trace=trace, tmpdir=tmpdir
    )
    full = np.empty((B, C, HW), dtype=np.float32)
    for c in range(NCORES):
        b, h = divmod(c, NCORES // B)
        full[b][:, h * I:(h + 1) * I] = np.asarray(
            res.results[c]["out"]).astype(np.float32)
    return full.reshape(B, C, 64, 64), res


def kernel(**inputs):
    out, _ = run(trace=False, **inputs)
    return out


# revision 51
# speedup vs baseline: 1.1166x; 1.0391x over previous
"""AttnBlock2D (B=4, C=512, H=W=64) on 8 Trainium2 NeuronCores.

Strategy: data-parallel over batch x sequence-parallel over output tokens.
Core c handles image b = c//2 and output-token half h = c%2.  The host
permutes the token axis per core (own half first), so the SPMD program is
identical on every core.

Math (scores-transposed formulation, softmax axis j on partitions).  With
M = Wk^T Wq and u = Wq^T bk, the softmax-relevant part of the score is

    score[i,j] = x_i^T M x_j + u . x_j  = g_i . x_j,   g_i = M^T x_i + u

(bq and all i-only terms cancel in softmax).  So the kernel computes

    g           = M^T x + u            (GEMM over the core's i-half ONLY;
                                        the j-side operand is raw x)
    e^T[j, i]   = exp(SCALE * g_i . x_j)     (ScalarE, bf16 out)
    acc[p, i]   = sum_jc e^T[jc*128+p, i]    (DVE partial-sum, f32)
    s[i]        = 1^T @ acc                  (one matmul per i-block)
    u[c, i]     = sum_j vT'[j, c] e^T[j, i]  (lhsT = vT', rhs = e^T)
    y[c, i]     = u[c, i] / s[i]

with vT'[j, c] = ((Wo Wv) x_j)_c + (Wo bv + bo)_c: softmax weights sum to
one, so the whole output bias (and Wo itself) ride inside vT and the
epilogue is a single normalising multiply.  All matmul operands are
bfloat16 (full PE rate at any free dim, half the DMA/SBUF of f32);
accumulation stays f32.

Schedule: warm-up matmuls on a memset scratch absorb the PE p-state ramp
during the initial weight DMA; apply matmuls trail the scores by a few
j-chunks so each i-block's epilogue (sum/recip/broadcast/mul/store) hides
under the next block's score matmuls; final i-blocks narrow down
(512,512,512,256,128,128) so the last epilogue tail is short.
"""

import numpy as np

import concourse.bass as bass  # noqa: F401  (engine types via nc.*)
import concourse.tile as tile
import concourse.mybir as mybir
from concourse import bacc
from concourse.alu_op_type import AluOpType
from concourse.bass_utils import run_bass_kernel_spmd

B = 4
C = 512            # C_IN == C_HID
HW = 64 * 64       # tokens per image
NCORES = 8
I = HW * B // NCORES   # 2048 output tokens per core

CK = 128           # partition chunk
NB = 512           # x-tile token width
NCH = C // CK      # 4 channel chunks
NJB = HW // CK     # 32 j-chunks
NNB = HW // NB     # 8 x tiles
NIB_MINE = I // NB  # 4 x tiles in my half

BLOCKS = [512, 512, 512, 256, 128, 128]   # i-block widths (sum = I)
assert sum(BLOCKS) == I

F32 = mybir.dt.float32
F32R = mybir.dt.float32r
BF16 = mybir.dt.bfloat16
AF = mybir.ActivationFunctionType
SCALE = 1.0 / float(np.sqrt(float(C)))

N_WARMUP = 12      # dummy matmuls riding out the first weight DMA


def build_bass():
    nc = bacc.Bacc(
        "TRN2", target_bir_lowering=False, debug=False, enable_asserts=False
    )

    xb = nc.dram_tensor("xb", [C, HW], BF16, kind="ExternalInput").ap()
    wqT = nc.dram_tensor("wqT", [C, C], BF16, kind="ExternalInput").ap()
    wvT = nc.dram_tensor("wvT", [C, C], BF16, kind="ExternalInput").ap()
    ub = nc.dram_tensor("ub", [CK, NCH], F32, kind="ExternalInput").ap()
    borow = nc.dram_tensor("borow", [1, C], F32, kind="ExternalInput").ap()
    out = nc.dram_tensor("out", [C, I], BF16, kind="ExternalOutput").ap()

    x3 = xb.rearrange("(a p) n -> p a n", p=CK)     # [128, 4, 4096]
    wq3d = wqT.rearrange("(a p) n -> p a n", p=CK)  # [128, 4, 512]
    wv3d = wvT.rearrange("(a p) n -> p a n", p=CK)
    out3 = out.rearrange("(a p) n -> p a n", p=CK)  # [128, 4, 2048]

    with tile.TileContext(nc) as tc:
        with tc.tile_pool(name="persist", bufs=1) as persist, \
             tc.tile_pool(name="ep", bufs=16) as ep, \
             tc.tile_pool(name="accp", bufs=2) as accp, \
             tc.tile_pool(name="rp", bufs=2) as rp, \
             tc.tile_pool(name="ftp", bufs=2) as ftp, \
             tc.tile_pool(name="psA", bufs=3, space="PSUM") as psA, \
             tc.tile_pool(name="psS", bufs=1, space="PSUM") as psS, \
             tc.tile_pool(name="psO", bufs=1, space="PSUM") as psO:

            # ---- persistent SBUF state ----
            x = persist.tile([CK, NCH, HW], BF16, name="x")   # all tokens
            g = persist.tile([CK, NCH, I], BF16, name="g")    # my i-half
            vT = persist.tile([CK, NJB, C], BF16, name="vT")
            ubias = persist.tile([CK, NCH], F32, name="ubias")
            bo_b = persist.tile([CK, C], F32, name="bo_b")
            bo_r = persist.tile([1, C], F32, name="bo_r")
            ones128 = persist.tile([CK, 1], F32, name="ones128")
            wq3 = persist.tile([CK, NCH, C], BF16, name="wq3")
            wv3 = persist.tile([CK, NCH, C], BF16, name="wv3")
            scratch = persist.tile([CK, NB], F32, name="scratch")

            # ---- input DMAs, first-needed first per queue ----
            # scalar queue: wq (first g matmul), then odd x tiles
            nc.scalar.dma_start(out=wq3, in_=wq3d)
            # sync queue: x tile nb=0, then even tiles
            nc.sync.dma_start(out=x[:, :, 0:NB], in_=x3[:, :, 0:NB])
            # gpsimd queue: small tensors, then wv (phase B's v GEMMs)
            nc.gpsimd.dma_start(out=ubias, in_=ub)
            nc.gpsimd.dma_start(out=bo_r, in_=borow)
            for ci in range(NCH):
                nc.gpsimd.dma_start(out=wv3[:, ci, :], in_=wv3d[:, ci, :])
            for nb in range(1, NNB):
                eng = nc.sync if nb % 2 == 0 else nc.scalar
                eng.dma_start(out=x[:, :, nb * NB:(nb + 1) * NB],
                              in_=x3[:, :, nb * NB:(nb + 1) * NB])

            # ---- warm-up: keep PE busy (and the p-state ramp burning)
            # while the first weight/x DMAs land; results land in a PSUM
            # bank that the first real accumulation group overwrites ----
            nc.vector.memset(scratch, 0.0)
            nc.vector.memset(ones128, 1.0)
            nc.gpsimd.partition_broadcast(bo_b, bo_r)
            psD = psS.tile([CK, NB], F32, name="psD", tag="sum",
                           space="PSUM")
            scr_r = scratch.bitcast(F32R)
            for i in range(N_WARMUP):
                # one accumulation group: no inter-matmul semaphores
                nc.tensor.matmul(psD, lhsT=scr_r[:, 0:CK], rhs=scr_r,
                                 start=(i == 0), stop=(i == N_WARMUP - 1))

            # ---- phase B: g over my half first (needs only wq + x0..3,
            # matching DMA arrival order), then vT' for all tokens ----
            for nb in range(NIB_MINE):
                for cc in range(NCH):
                    pg = psA.tile([CK, NB], F32, name="pg", tag="psA",
                                  space="PSUM")
                    for ci in range(NCH):
                        nc.tensor.matmul(
                            pg,
                            lhsT=wq3[:, ci, cc * CK:(cc + 1) * CK],
                            rhs=x[:, ci, nb * NB:(nb + 1) * NB],
                            start=(ci == 0), stop=(ci == NCH - 1),
                        )
                    # g = M^T x + u; bq cancels in softmax
                    nc.scalar.activation(
                        g[:, cc, nb * NB:(nb + 1) * NB], pg, AF.Identity,
                        bias=ubias[:, cc:cc + 1])
            for jc in range(NJB):
                pv = psA.tile([CK, C], F32, name="pv", tag="psA",
                              space="PSUM")
                for ci in range(NCH):
                    nc.tensor.matmul(
                        pv,
                        lhsT=x[:, ci, jc * CK:(jc + 1) * CK],
                        rhs=wv3[:, ci, :],
                        start=(ci == 0), stop=(ci == NCH - 1),
                    )
                # vT' = (WoWv)x + (Wo bv + bo): softmax weights sum to
                # 1, so the output bias rides inside vT
                nc.vector.tensor_tensor(out=vT[:, jc, :], in0=pv,
                                        in1=bo_b, op=AluOpType.add)

            # ---- phase C: attention per i-block, software-pipelined ----
            offs = np.cumsum([0] + BLOCKS).tolist()

            state: dict = {}
            prev: dict = {}
            pending: list = []    # deferred closures from the previous block

            def scores_chunk(jc):
                w, off = state["w"], state["off"]
                ps_ = psA.tile([CK, w], F32, name="ps", tag="psA",
                               space="PSUM")
                for ci in range(NCH):
                    nc.tensor.matmul(
                        ps_,
                        lhsT=x[:, ci, jc * CK:(jc + 1) * CK],
                        rhs=g[:, ci, off:off + w],
                        start=(ci == 0), stop=(ci == NCH - 1),
                    )
                et = ep.tile([CK, w], BF16, name="et", tag="et")
                nc.scalar.activation(et, ps_, AF.Exp, scale=SCALE)
                acc = state["acc"][:, 0:w]
                if jc == 0:
                    nc.vector.tensor_copy(acc, et)
                else:
                    # alternate engines: DVE also runs the epilogue muls
                    # (gpsimd cannot touch PSUM), so share the adds
                    eng = nc.vector if jc % 2 == 0 else nc.gpsimd
                    eng.tensor_add(acc, acc, et)
                state["ets"][jc] = et

            def make_apply(st, jc):
                def go():
                    et = st["ets"][jc]
                    for cc in range(NCH):
                        nc.tensor.matmul(
                            st["po"][cc],
                            lhsT=vT[:, jc, cc * CK:(cc + 1) * CK],
                            rhs=et,
                            start=(jc == 0), stop=(jc == NJB - 1),
                        )
                return go

            def make_sum_chain(st):
                def go():
                    w = st["w"]
                    psum = psS.tile([1, w], F32, name="psum", tag="sum",
                                    space="PSUM")
                    nc.tensor.matmul(psum, lhsT=ones128.bitcast(F32R),
                                     rhs=st["acc"][:, 0:w],
                                     start=True, stop=True)
                    r1 = rp.tile([1, NB], F32, name="r1", tag="r1")
                    nc.vector.reciprocal(r1[:, 0:w], psum)
                    rb = rp.tile([CK, NB], F32, name="rb", tag="rb")
                    nc.gpsimd.partition_broadcast(rb[:, 0:w], r1[:, 0:w])
                    st["rb"] = rb
                return go

            def make_epilogue(st):
                def go():
                    w, off, rb = st["w"], st["off"], st["rb"]
                    ft = ftp.tile([CK, NCH, NB], BF16, name="ft", tag="ft")
                    for cc in range(NCH):
                        # must be DVE: gpsimd cannot read PSUM
                        nc.vector.tensor_tensor(out=ft[:, cc, 0:w],
                                                in0=st["po"][cc],
                                                in1=rb[:, 0:w],
                                                op=AluOpType.mult)
                    # one fused store: a single HWDGE generation on the
                    # shared DGE device instead of four
                    nc.sync.dma_start(out=out3[:, :, off:off + w],
                                      in_=ft[:, :, 0:w])
                return go

            # per-width pipeline knobs: narrow blocks have shorter score
            # chunks, so the fixed-latency epilogue/exp chains need more
            # slots of cover before the applies start
            KNOBS = {512: (6, 3), 256: (9, 5), 128: (14, 9)}
            for k, w in enumerate(BLOCKS):
                start_slot, lag = KNOBS[w]
                state = {
                    "w": w,
                    "off": offs[k],
                    "acc": accp.tile([CK, NB], F32R, name="acc", tag="acc"),
                    "po": [psO.tile([CK, w], F32, name=f"po{cc}",
                                    tag=f"po{cc}", space="PSUM")
                           for cc in range(NCH)],
                    "ets": [None] * NJB,
                }
                nxt = 0          # next own apply chunk to emit
                for jc in range(NJB):
                    scores_chunk(jc)
                    if pending:
                        pending.pop(0)()
                        if not pending and prev:
                            # prev block's po complete: sum/recip/bcast were
                            # emitted at slot 1; normalising muls go now so
                            # they run under the scores-only slots below
                            make_epilogue(prev)()
                    elif jc >= start_slot:
                        # catch up two per slot, then settle at lag
                        for _ in range(2):
                            if nxt <= jc - lag:
                                make_apply(state, nxt)()
                                nxt += 1
                    if jc == 1 and prev:
                        make_sum_chain(prev)()
                # defer the last `lag` apply chunks into the next block
                pending = [make_apply(state, t) for t in range(nxt, NJB)]
                prev = state

            # tail: drain the last block
            for fn in pending:
                fn()
            make_sum_chain(prev)()
            make_epilogue(prev)()

    nc.compile()
    return nc


_NC = None


def _get_nc():
    global _NC
    if _NC is None:
        _NC = build_bass()
    return _NC


def _make_in_maps(inp, Wk, bk, Wq, bq, Wv, bv, Wo, bo):
    import ml_dtypes

    bf = ml_dtypes.bfloat16
    x_all = np.asarray(inp, dtype=np.float32).reshape(B, C, HW)
    # M = Wk^T Wq: lhsT of the g GEMM (g = M^T x + u)
    wqT = np.ascontiguousarray(
        (np.asarray(Wk, np.float64).T @ np.asarray(Wq, np.float64))
    ).astype(bf)
    wvT = np.ascontiguousarray(
        (np.asarray(Wo, np.float64) @ np.asarray(Wv, np.float64)).T
    ).astype(bf)
    u = (np.asarray(Wq, np.float64).T @ np.asarray(bk, np.float64))
    ub = np.ascontiguousarray(
        u.astype(np.float32).reshape(NCH, CK).T)
    bo_eff = (np.asarray(Wo, np.float32) @ np.asarray(bv, np.float32)
              + np.asarray(bo, np.float32))
    borow = np.ascontiguousarray(bo_eff.reshape(1, C).astype(np.float32))

    in_maps = []
    for c in range(NCORES):
        b, h = divmod(c, NCORES // B)
        xb = x_all[b]
        if h == 1:      # own half first; attention is j-permutation-invariant
            xb = np.concatenate([xb[:, I:], xb[:, :I]], axis=1)
        in_maps.append({
            "xb": np.ascontiguousarray(xb).astype(bf),
            "wqT": wqT, "wvT": wvT, "ub": ub, "borow": borow,
        })
    return in_maps


def run(trace=False, tmpdir=None, **inputs):
    nc = _get_nc()
    in_maps = _make_in_maps(**inputs)
    res = run_bass_kernel_spmd(
        nc, in_maps, core_ids=list(range(NCORES)), trace=trace, tmpdir=tmpdir
    )
    full = np.empty((B, C, HW), dtype=np.float32)
    for c in range(NCORES):
        b, h = divmod(c, NCORES // B)
        full[b][:, h * I:(h + 1) * I] = np.asarray(
            res.results[c]["out"]).astype(np.float32)
    return full.reshape(B, C, 64, 64), res


def kernel(**inputs):
    out, _ = run(trace=False, **inputs)
    return out


# revision 66
# speedup vs baseline: 1.1181x; 1.0013x over previous
"""AttnBlock2D (B=4, C=512, H=W=64) on 8 Trainium2 NeuronCores.

Strategy: data-parallel over batch x sequence-parallel over output tokens.
Core c handles image b = c//2 and output-token half h = c%2.  The host
permutes the token axis per core (own half first), so the SPMD program is
identical on every core.

Math (scores-transposed formulation, softmax axis j on partitions).  With
M = Wk^T Wq and u = Wq^T bk, the softmax-relevant part of the score is

    score[i,j] = x_i^T M x_j + u . x_j  = g_i . x_j,   g_i = M^T x_i + u

(bq and all i-only terms cancel in softmax).  So the kernel computes

    g           = M^T x + u            (GEMM over the core's i-half ONLY;
                                        the j-side operand is raw x)
    e^T[j, i]   = exp(SCALE * g_i . x_j)     (ScalarE, bf16 out)
    acc[p, i]   = sum_jc e^T[jc*128+p, i]    (DVE partial-sum, f32)
    s[i]        = 1^T @ acc                  (one matmul per i-block)
    u[c, i]     = sum_j vT'[j, c] e^T[j, i]  (lhsT = vT', rhs = e^T)
    y[c, i]     = u[c, i] / s[i]

with vT'[j, c] = ((Wo Wv) x_j)_c + (Wo bv + bo)_c: softmax weights sum to
one, so the whole output bias (and Wo itself) ride inside vT and the
epilogue is a single normalising multiply.  All matmul operands are
bfloat16 (full PE rate at any free dim, half the DMA/SBUF of f32);
accumulation stays f32.

Schedule: warm-up matmuls on a memset scratch absorb the PE p-state ramp
during the initial weight DMA; apply matmuls trail the scores by a few
j-chunks so each i-block's epilogue (sum/recip/broadcast/mul/store) hides
under the next block's score matmuls; final i-blocks narrow down
(512,512,512,256,128,128) so the last epilogue tail is short.
"""

import numpy as np

import concourse.bass as bass  # noqa: F401  (engine types via nc.*)
import concourse.tile as tile
import concourse.mybir as mybir
from concourse import bacc
from concourse.alu_op_type import AluOpType
from concourse.bass_utils import run_bass_kernel_spmd

B = 4
C = 512            # C_IN == C_HID
HW = 64 * 64       # tokens per image
NCORES = 8
I = HW * B // NCORES   # 2048 output tokens per core

CK = 128           # partition chunk
NB = 512           # x-tile token width
NCH = C // CK      # 4 channel chunks
NJB = HW // CK     # 32 j-chunks
NNB = HW // NB     # 8 x tiles
NIB_MINE = I // NB  # 4 x tiles in my half

BLOCKS = [512, 512, 512, 256, 128, 128]   # i-block widths (sum = I)
assert sum(BLOCKS) == I

F32 = mybir.dt.float32
F32R = mybir.dt.float32r
BF16 = mybir.dt.bfloat16
AF = mybir.ActivationFunctionType
SCALE = 1.0 / float(np.sqrt(float(C)))

N_WARMUP = 12      # dummy matmuls riding out the first weight DMA


def build_bass():
    nc = bacc.Bacc(
        "TRN2", target_bir_lowering=False, debug=False, enable_asserts=False
    )

    xb = nc.dram_tensor("xb", [C, HW], BF16, kind="ExternalInput").ap()
    wqT = nc.dram_tensor("wqT", [C, C], BF16, kind="ExternalInput").ap()
    wvT = nc.dram_tensor("wvT", [C, C], BF16, kind="ExternalInput").ap()
    ub = nc.dram_tensor("ub", [CK, NCH], F32, kind="ExternalInput").ap()
    borow = nc.dram_tensor("borow", [1, C], F32, kind="ExternalInput").ap()
    # out packed [p, i, a] so every store is per-partition contiguous (no
    # sub-512B descriptor penalty); host unpacks
    out = nc.dram_tensor("out", [CK, I, NCH], BF16,
                         kind="ExternalOutput").ap()

    x3 = xb.rearrange("(a p) n -> p a n", p=CK)     # [128, 4, 4096]
    wq3d = wqT.rearrange("(a p) n -> p a n", p=CK)  # [128, 4, 512]
    wv3d = wvT.rearrange("(a p) n -> p a n", p=CK)

    with tile.TileContext(nc) as tc:
        with tc.tile_pool(name="persist", bufs=1) as persist, \
             tc.tile_pool(name="ep", bufs=16) as ep, \
             tc.tile_pool(name="accp", bufs=2) as accp, \
             tc.tile_pool(name="rp", bufs=2) as rp, \
             tc.tile_pool(name="ftp", bufs=2) as ftp, \
             tc.tile_pool(name="psA", bufs=3, space="PSUM") as psA, \
             tc.tile_pool(name="psS", bufs=1, space="PSUM") as psS, \
             tc.tile_pool(name="psO", bufs=1, space="PSUM") as psO:

            # ---- persistent SBUF state ----
            x = persist.tile([CK, NCH, HW], BF16, name="x")   # all tokens
            g = persist.tile([CK, NCH, I], BF16, name="g")    # my i-half
            vT = persist.tile([CK, NJB, C], BF16, name="vT")
            ubias = persist.tile([CK, NCH], F32, name="ubias")
            bo_b = persist.tile([CK, C], F32, name="bo_b")
            bo_r = persist.tile([1, C], F32, name="bo_r")
            ones128 = persist.tile([CK, 1], F32, name="ones128")
            wq3 = persist.tile([CK, NCH, C], BF16, name="wq3")
            wv3 = persist.tile([CK, NCH, C], BF16, name="wv3")
            scratch = persist.tile([CK, NB], F32, name="scratch")

            # ---- input DMAs, first-needed first per queue ----
            # scalar queue: wq (first g matmul), then odd x tiles
            nc.scalar.dma_start(out=wq3, in_=wq3d)
            # sync queue: x tile nb=0, then even tiles
            nc.sync.dma_start(out=x[:, :, 0:NB], in_=x3[:, :, 0:NB])
            # gpsimd queue: small tensors, then wv (phase B's v GEMMs)
            nc.gpsimd.dma_start(out=ubias, in_=ub)
            nc.gpsimd.dma_start(out=bo_r, in_=borow)
            for ci in range(NCH):
                nc.gpsimd.dma_start(out=wv3[:, ci, :], in_=wv3d[:, ci, :])
            for nb in range(1, NNB):
                eng = nc.sync if nb % 2 == 0 else nc.scalar
                eng.dma_start(out=x[:, :, nb * NB:(nb + 1) * NB],
                              in_=x3[:, :, nb * NB:(nb + 1) * NB])

            # ---- warm-up: keep PE busy (and the p-state ramp burning)
            # while the first weight/x DMAs land; results land in a PSUM
            # bank that the first real accumulation group overwrites ----
            nc.vector.memset(scratch, 0.0)
            nc.vector.memset(ones128, 1.0)
            nc.gpsimd.partition_broadcast(bo_b, bo_r)
            psD = psS.tile([CK, NB], F32, name="psD", tag="sum",
                           space="PSUM")
            scr_r = scratch.bitcast(F32R)
            for i in range(N_WARMUP):
                # one accumulation group: no inter-matmul semaphores
                nc.tensor.matmul(psD, lhsT=scr_r[:, 0:CK], rhs=scr_r,
                                 start=(i == 0), stop=(i == N_WARMUP - 1))

            # ---- phase B: g over my half first (needs only wq + x0..3,
            # matching DMA arrival order), then vT' for all tokens ----
            for nb in range(NIB_MINE):
                for cc in range(NCH):
                    pg = psA.tile([CK, NB], F32, name="pg", tag="psA",
                                  space="PSUM")
                    for ci in range(NCH):
                        nc.tensor.matmul(
                            pg,
                            lhsT=wq3[:, ci, cc * CK:(cc + 1) * CK],
                            rhs=x[:, ci, nb * NB:(nb + 1) * NB],
                            start=(ci == 0), stop=(ci == NCH - 1),
                        )
                    # g = M^T x + u; bq cancels in softmax
                    nc.scalar.activation(
                        g[:, cc, nb * NB:(nb + 1) * NB], pg, AF.Identity,
                        bias=ubias[:, cc:cc + 1])
            for jc in range(NJB):
                pv = psA.tile([CK, C], F32, name="pv", tag="psA",
                              space="PSUM")
                for ci in range(NCH):
                    nc.tensor.matmul(
                        pv,
                        lhsT=x[:, ci, jc * CK:(jc + 1) * CK],
                        rhs=wv3[:, ci, :],
                        start=(ci == 0), stop=(ci == NCH - 1),
                    )
                # vT' = (WoWv)x + (Wo bv + bo): softmax weights sum to
                # 1, so the output bias rides inside vT
                nc.vector.tensor_tensor(out=vT[:, jc, :], in0=pv,
                                        in1=bo_b, op=AluOpType.add)

            # ---- phase C: attention per i-block, software-pipelined ----
            offs = np.cumsum([0] + BLOCKS).tolist()

            state: dict = {}
            prev: dict = {}
            pending: list = []    # deferred closures from the previous block

            def scores_chunk(jc):
                w, off = state["w"], state["off"]
                ps_ = psA.tile([CK, w], F32, name="ps", tag="psA",
                               space="PSUM")
                for ci in range(NCH):
                    nc.tensor.matmul(
                        ps_,
                        lhsT=x[:, ci, jc * CK:(jc + 1) * CK],
                        rhs=g[:, ci, off:off + w],
                        start=(ci == 0), stop=(ci == NCH - 1),
                    )
                et = ep.tile([CK, w], BF16, name="et", tag="et")
                nc.scalar.activation(et, ps_, AF.Exp, scale=SCALE)
                acc = state["acc"][:, 0:w]
                if jc == 0:
                    nc.vector.tensor_copy(acc, et)
                else:
                    # alternate engines: DVE also runs the epilogue muls
                    # (gpsimd cannot touch PSUM), so share the adds
                    eng = nc.vector if jc % 2 == 0 else nc.gpsimd
                    eng.tensor_add(acc, acc, et)
                state["ets"][jc] = et

            def make_apply(st, jc):
                def go():
                    et = st["ets"][jc]
                    for cc in range(NCH):
                        nc.tensor.matmul(
                            st["po"][cc],
                            lhsT=vT[:, jc, cc * CK:(cc + 1) * CK],
                            rhs=et,
                            start=(jc == 0), stop=(jc == NJB - 1),
                        )
                return go

            def make_sum_chain(st):
                def go():
                    w = st["w"]
                    psum = psS.tile([1, w], F32, name="psum", tag="sum",
                                    space="PSUM")
                    nc.tensor.matmul(psum, lhsT=ones128.bitcast(F32R),
                                     rhs=st["acc"][:, 0:w],
                                     start=True, stop=True)
                    r1 = rp.tile([1, NB], F32, name="r1", tag="r1")
                    nc.vector.reciprocal(r1[:, 0:w], psum)
                    rb = rp.tile([CK, NB], F32, name="rb", tag="rb")
                    nc.gpsimd.partition_broadcast(rb[:, 0:w], r1[:, 0:w])
                    st["rb"] = rb
                return go

            def make_epilogue(st):
                def go():
                    w, off, rb = st["w"], st["off"], st["rb"]
                    ft = ftp.tile([CK, NB, NCH], BF16, name="ft", tag="ft")
                    for cc in range(NCH):
                        # must be DVE: gpsimd cannot read PSUM
                        nc.vector.tensor_tensor(out=ft[:, 0:w, cc],
                                                in0=st["po"][cc],
                                                in1=rb[:, 0:w],
                                                op=AluOpType.mult)
                    # one fused store: a single HWDGE generation on the
                    # shared DGE device instead of four
                    nc.sync.dma_start(out=out[:, off:off + w, :],
                                      in_=ft[:, 0:w, :])
                return go

            # per-width pipeline knobs: narrow blocks have shorter score
            # chunks, so the fixed-latency epilogue/exp chains need more
            # slots of cover before the applies start
            KNOBS = {512: (6, 3), 256: (9, 5), 128: (14, 9)}
            for k, w in enumerate(BLOCKS):
                start_slot, lag = KNOBS[w]
                state = {
                    "w": w,
                    "off": offs[k],
                    "acc": accp.tile([CK, NB], F32R, name="acc", tag="acc"),
                    "po": [psO.tile([CK, w], F32, name=f"po{cc}",
                                    tag=f"po{cc}", space="PSUM")
                           for cc in range(NCH)],
                    "ets": [None] * NJB,
                }
                nxt = 0          # next own apply chunk to emit
                for jc in range(NJB):
                    scores_chunk(jc)
                    if pending:
                        pending.pop(0)()
                        if not pending and prev:
                            # prev block's po complete: sum/recip/bcast were
                            # emitted at slot 1; normalising muls go now so
                            # they run under the scores-only slots below
                            make_epilogue(prev)()
                    elif jc >= start_slot:
                        # catch up two per slot, then settle at lag
                        for _ in range(2):
                            if nxt <= jc - lag:
                                make_apply(state, nxt)()
                                nxt += 1
                    if jc == 1 and prev:
                        make_sum_chain(prev)()
                # defer the last `lag` apply chunks into the next block
                pending = [make_apply(state, t) for t in range(nxt, NJB)]
                prev = state

            # tail: drain the last block
            for fn in pending:
                fn()
            make_sum_chain(prev)()
            make_epilogue(prev)()

    nc.compile()
    return nc


_NC = None


def _get_nc():
    global _NC
    if _NC is None:
        _NC = build_bass()
    return _NC


def _make_in_maps(inp, Wk, bk, Wq, bq, Wv, bv, Wo, bo):
    import ml_dtypes

    bf = ml_dtypes.bfloat16
    x_all = np.asarray(inp, dtype=np.float32).reshape(B, C, HW)
    # M = Wk^T Wq: lhsT of the g GEMM (g = M^T x + u)
    wqT = np.ascontiguousarray(
        (np.asarray(Wk, np.float64).T @ np.asarray(Wq, np.float64))
    ).astype(bf)
    wvT = np.ascontiguousarray(
        (np.asarray(Wo, np.float64) @ np.asarray(Wv, np.float64)).T
    ).astype(bf)
    u = (np.asarray(Wq, np.float64).T @ np.asarray(bk, np.float64))
    ub = np.ascontiguousarray(
        u.astype(np.float32).reshape(NCH, CK).T)
    bo_eff = (np.asarray(Wo, np.float32) @ np.asarray(bv, np.float32)
              + np.asarray(bo, np.float32))
    borow = np.ascontiguousarray(bo_eff.reshape(1, C).astype(np.float32))

    in_maps = []
    for c in range(NCORES):
        b, h = divmod(c, NCORES // B)
        xb = x_all[b]
        if h == 1:      # own half first; attention is j-permutation-invariant
            xb = np.concatenate([xb[:, I:], xb[:, :I]], axis=1)
        in_maps.append({
            "xb": np.ascontiguousarray(xb).astype(bf),
            "wqT": wqT, "wvT": wvT, "ub": ub, "borow": borow,
        })
    return in_maps


def run(trace=False, tmpdir=None, **inputs):
    nc = _get_nc()
    in_maps = _make_in_maps(**inputs)
    res = run_bass_kernel_spmd(
        nc, in_maps, core_ids=list(range(NCORES)), trace=trace, tmpdir=tmpdir
    )
    full = np.empty((B, C, HW), dtype=np.float32)
    for c in range(NCORES):
        b, h = divmod(c, NCORES // B)
        # device layout [p, i, a] -> channel c = a*CK + p
        o = np.asarray(res.results[c]["out"]).astype(np.float32)
        full[b][:, h * I:(h + 1) * I] = o.transpose(2, 0, 1).reshape(C, I)
    return full.reshape(B, C, 64, 64), res


def kernel(**inputs):
    out, _ = run(trace=False, **inputs)
    return out
